# revision 2
# baseline (speedup 1.0000x reference)
"""Trainium2 Bass kernel for nn_Decoder: 2-layer LSTM + vocab-32000 greedy decoder.

Strategy (8 NeuronCores, one trn2 chip):
- Everything fp32 on the PE (exact argmax tracking vs the fp32 reference).
- All matvecs run "weights-moving": stationary = h columns [128,1], moving =
  W^T chunks streamed at N columns/instr, 4-way col-tiled for 4x concurrency.
- fc weight [32768, 1024] sharded by vocab across cores (4096 rows/core,
  resident in SBUF).  LSTM hidden state sharded 128 units/core; per step the
  h-slices are all-gathered via remote SBUF-to-SBUF DMA (XOR slot pattern),
  the per-core argmax candidates likewise.  3 small exchanges per step.
- Single NEFF, 1024-iteration For_i loop x 2 unrolled steps = 2048 steps.
"""
import numpy as np

import concourse.bass as bass
import concourse.mybir as mybir
import concourse.tile as tile
from concourse import bacc
from concourse.bass import _add_dep_helper
from concourse.masks import make_identity

F32 = mybir.dt.float32
U32 = mybir.dt.uint32
AF = mybir.ActivationFunctionType
ALU = mybir.AluOpType

H = 1024
V = 32000
VPAD = 32768
VLOC = VPAD // 8          # 4096 vocab rows per core
L = 2048
NCORES = 8
# logical -> physical NeuronCore map observed on this trn2 chip (involution).
PERM = [0, 1, 2, 3, 6, 7, 4, 5]
BIGVAL = 65536.0
ABLATE = set()  # > any vocab idx; keeps idx arithmetic exact in fp32

# torch gate row order in the 4H weights: i, f, g, o.
# col-tile j holds gate type: j=0 -> i, 1 -> f, 2 -> o, 3 -> g
GATE_OFF = [0, H, 3 * H, 2 * H]  # row offset of gate-type j in the 4H dim


def eff_src(r, c):
    """Logical id of the core whose data lands in receiver r's slot c."""
    return PERM[PERM[r] ^ c]


def build_decoder(n_iters):
    """Build the SPMD program. n_iters loop iterations x 2 steps each."""
    nc = bacc.Bacc(None, num_devices=NCORES, detect_race_conditions=False)

    wfc_d = nc.dram_tensor("wfc", [128, 8 * VLOC], F32, kind="ExternalInput")
    hh0_d = nc.dram_tensor("hh0", [128, 8 * 512], F32, kind="ExternalInput")
    ih1_d = nc.dram_tensor("ih1", [128, 8 * 512], F32, kind="ExternalInput")
    hh1_d = nc.dram_tensor("hh1", [128, 8 * 512], F32, kind="ExternalInput")
    wih0_d = nc.dram_tensor("wih0", [1, 512], F32, kind="ExternalInput")
    b0_d = nc.dram_tensor("b0", [1, 512], F32, kind="ExternalInput")
    b1_d = nc.dram_tensor("b1", [1, 512], F32, kind="ExternalInput")
    bfc_d = nc.dram_tensor("bfc", [1, VLOC], F32, kind="ExternalInput")
    base_d = nc.dram_tensor("base", [128, 1], F32, kind="ExternalInput")
    h0i_d = nc.dram_tensor("h0init", [128, 8], F32, kind="ExternalInput")
    h1i_d = nc.dram_tensor("h1init", [128, 8], F32, kind="ExternalInput")
    c0i_d = nc.dram_tensor("c0init", [128, 1], F32, kind="ExternalInput")
    c1i_d = nc.dram_tensor("c1init", [128, 1], F32, kind="ExternalInput")
    x0_d = nc.dram_tensor("x0", [1, 1], F32, kind="ExternalInput")
    out_d = nc.dram_tensor("out", [2 * n_iters + 1, VLOC], F32,
                           kind="ExternalOutput")

    h0_sem = nc.alloc_semaphore("h0_sem")
    h1_sem = nc.alloc_semaphore("h1_sem")
    cd_sem = nc.alloc_semaphore("cd_sem")
    lsem = nc.alloc_semaphore("lsem")
    nc.add_non_barrier_sems([h0_sem.num, h1_sem.num, cd_sem.num, lsem.num])

    r_h0 = nc.tensor.alloc_register("r_h0")
    r_h1 = nc.tensor.alloc_register("r_h1")
    r_cd = nc.vector.alloc_register("r_cd")

    post_waits = []   # (instruction, sem, register)
    P32 = slice(0, 97, 32)   # partitions {0,32,64,96}

    with tile.TileContext(nc) as tc:
        with tc.tile_pool(name="wts", bufs=1) as wp, \
             tc.tile_pool(name="st", bufs=1) as sp, \
             tc.tile_pool(name="ps", bufs=1, space="PSUM") as pp:

            wfc = wp.tile([128, 8 * VLOC], F32, tag="wfc")
            hh0 = wp.tile([128, 8 * 512], F32, tag="hh0")
            ih1 = wp.tile([128, 8 * 512], F32, tag="ih1")
            hh1 = wp.tile([128, 8 * 512], F32, tag="hh1")
            wih0 = wp.tile([1, 512], F32, tag="wih0")
            b0 = wp.tile([1, 512], F32, tag="b0")
            b1 = wp.tile([1, 512], F32, tag="b1")
            bfc = wp.tile([1, VLOC], F32, tag="bfc")
            base = wp.tile([128, 1], F32, tag="base")
            ident = wp.tile([128, 128], F32, tag="ident")
            one = wp.tile([1, 1], F32, tag="one")
            big4 = wp.tile([1, 4], F32, tag="big4")
            big8 = wp.tile([1, 8], F32, tag="big8")
            x_s = wp.tile([1, 1], F32, tag="x")
            c0 = wp.tile([128, 1], F32, tag="c0")
            gcol_s = wp.tile([128, 1], F32, tag="gcol")
            c1 = wp.tile([128, 1], F32, tag="c1")
            h0buf = [wp.tile([128, 8], F32, tag=f"h0buf{p}", name=f"h0buf{p}")
                     for p in range(2)]
            h1buf = [wp.tile([128, 8], F32, tag=f"h1buf{p}", name=f"h1buf{p}")
                     for p in range(2)]
            cdbuf = [wp.tile([128, 16], F32, tag=f"cdbuf{p}", name=f"cdbuf{p}")
                     for p in range(2)]

            for dst, src in ((wfc, wfc_d), (hh0, hh0_d), (ih1, ih1_d),
                             (hh1, hh1_d), (wih0, wih0_d), (b0, b0_d),
                             (b1, b1_d), (bfc, bfc_d), (base, base_d),
                             (h0buf[1], h0i_d), (h1buf[1], h1i_d),
                             (c0, c0i_d), (c1, c1i_d), (x_s, x0_d)):
                nc.sync.dma_start(dst[:], src[:])
            make_identity(nc, ident[:])
            nc.vector.memset(h0buf[0][:], 0.0)
            nc.vector.memset(h1buf[0][:], 0.0)
            nc.vector.memset(cdbuf[0][:], 0.0)
            nc.vector.memset(cdbuf[1][:], 0.0)
            nc.vector.memset(one[:], 1.0)
            nc.vector.memset(big4[:], BIGVAL)
            nc.vector.memset(big8[:], BIGVAL)
            rm0 = nc.tensor.reg_mov(r_h0, 0)
            rm1 = nc.tensor.reg_mov(r_h1, 0)
            rm2 = nc.vector.reg_mov(r_cd, 0)

            # psum tiles (8 banks):
            g0_ps = pp.tile([128, 128], F32, tag="g0")
            g1_ps = pp.tile([128, 128], F32, tag="g1")
            tr_ps = pp.tile([128, 128], F32, tag="tr")
            fcA_ps = pp.tile([128, 512], F32, tag="fcA")
            fcB_ps = pp.tile([128, 512], F32, tag="fcB")
            ctv_ps = pp.tile([1, 128], F32, tag="ctv")
            cti_ps = pp.tile([1, 128], F32, tag="cti")
            for _pst in (g0_ps, g1_ps, fcA_ps, fcB_ps):
                nc.vector.memset(_pst[:], 0.0)

            state = {
                "pe_last": rm1, "dve_last": rm2,
                "prep_last": None, "trig_last": None,
            }

            def chain(engine_key, inst):
                prev = state[engine_key]
                if prev is not None:
                    _add_dep_helper(inst.ins, prev.ins, sync=False,
                                    reason=f"order {engine_key}")
                state[engine_key] = inst
                return inst

            def bcast7(buf, width, sem, src_ap):
                """7 broadcasts of src_ap into peers' buf slot k, then trigger."""
                if "comm" in ABLATE:
                    return None
                for k in range(1, 8):
                    rdests = [None] * 8
                    rdests[k] = (0, k)
                    pr = nc.gpsimd.remote_dma_broadcast(
                        buf[:, k * width:(k + 1) * width], src_ap,
                        sem, lsem, rdests=rdests)
                    chain("prep_last", pr)
                tg = nc.gpsimd.trigger_dma(count=7)
                chain("prep_last", tg)
                return tg

            def cell(l_idx, g_ps, gate_sb, c_st, th_t, t1, t2, hdst):
                """LSTM cell: gates psum [4p,128] -> h column [128,1]."""
                nc.scalar.activation(gate_sb[0:65, 0:128],
                                     g_ps[0:65, 0:128], AF.Sigmoid)
                nc.scalar.activation(gate_sb[96:97, 0:128],
                                     g_ps[96:97, 0:128], AF.Tanh)
                tr = nc.tensor.transpose(tr_ps[:], gate_sb[:], ident[:])
                chain("pe_last", tr)
                # cols after transpose: i@0, f@32, o@64, g@96
                nc.vector.tensor_copy(gcol_s[:], tr_ps[:, 96:97])
                nc.vector.tensor_tensor(t1[:], tr_ps[:, 0:1], gcol_s[:],
                                        ALU.mult)
                nc.vector.tensor_tensor(t2[:], tr_ps[:, 32:33], c_st[:],
                                        ALU.mult)
                nc.vector.tensor_tensor(c_st[:], t1[:], t2[:], ALU.add)
                nc.scalar.activation(th_t[:], c_st[:], AF.Tanh)
                nc.vector.tensor_tensor(hdst, tr_ps[:, 64:65], th_t[:],
                                        ALU.mult)

            def step(u, i_var):
                p, q = u, 1 - u
                stg = stgs[u]
                mx, mi, mif, gcand = mxs[u], mis[u], mifs[u], gcands[u]

                # ---- g0 = b0 + hh0 @ h0(q) + x*wih0
                for j in range(4):
                    mm = nc.tensor.matmul(
                        g0_ps[32 * j:32 * j + 1, 0:128], one[:],
                        b0[:, j * 128:(j + 1) * 128],
                        start=True, stop=False, tile_position=(0, 32 * j),
                        skip_group_check=True)
                    chain("pe_last", mm)
                for c in range(8):
                    for j in range(4):
                        mm = nc.tensor.matmul(
                            g0_ps[32 * j:32 * j + 1, 0:128],
                            h0buf[q][:, c:c + 1],
                            hh0[:, c * 512 + j * 128:c * 512 + (j + 1) * 128],
                            start=False, stop=False, tile_position=(0, 32 * j),
                            skip_group_check=True)
                        chain("pe_last", mm)
                for j in range(4):
                    mm = nc.tensor.matmul(
                        g0_ps[32 * j:32 * j + 1, 0:128], x_s[:],
                        wih0[:, j * 128:(j + 1) * 128],
                        start=False, stop=(j == 3), tile_position=(0, 32 * j),
                        skip_group_check=True)
                    chain("pe_last", mm)

                # ---- cell0 -> h0 column into slot 0 of h0buf[p], broadcast
                cell(0, g0_ps, gates_sb[u], c0, th_s[u], t1_s[u], t2_s[u],
                     h0buf[p][:, 0:1])
                bcast7(h0buf[p], 1, h0_sem, h0buf[p][:, 0:1])

                # ---- g1 = b1 + hh1 @ h1(q) + ih1 @ h0(p)
                for j in range(4):
                    mm = nc.tensor.matmul(
                        g1_ps[32 * j:32 * j + 1, 0:128], one[:],
                        b1[:, j * 128:(j + 1) * 128],
                        start=True, stop=False, tile_position=(0, 32 * j),
                        skip_group_check=True)
                    chain("pe_last", mm)
                for c in range(8):
                    for j in range(4):
                        mm = nc.tensor.matmul(
                            g1_ps[32 * j:32 * j + 1, 0:128],
                            h1buf[q][:, c:c + 1],
                            hh1[:, c * 512 + j * 128:c * 512 + (j + 1) * 128],
                            start=False, stop=False, tile_position=(0, 32 * j),
                            skip_group_check=True)
                        chain("pe_last", mm)
                ra = nc.tensor.reg_add(r_h0, r_h0, 14)
                chain("pe_last", ra)
                first = None
                for c in range(8):
                    for j in range(4):
                        mm = nc.tensor.matmul(
                            g1_ps[32 * j:32 * j + 1, 0:128],
                            h0buf[p][:, c:c + 1],
                            ih1[:, c * 512 + j * 128:c * 512 + (j + 1) * 128],
                            start=False, stop=(c == 7), tile_position=(0, 32 * j),
                            skip_group_check=True)
                        chain("pe_last", mm)
                        if first is None:
                            first = mm
                            if "comm" not in ABLATE:
                                post_waits.append((mm, h0_sem, r_h0))

                # ---- cell1 -> h1 column, broadcast
                cell(1, g1_ps, gates_sb2[u], c1, th2_s[u], t1_s[u], t2_s[u],
                     h1buf[p][:, 0:1])
                bcast7(h1buf[p], 1, h1_sem, h1buf[p][:, 0:1])

                # ---- fc = relu(bfc + Wfc @ h1(p)) ; two banks per col-tile
                ra1 = nc.tensor.reg_add(r_h1, r_h1, 14)
                chain("pe_last", ra1)
                for bi, fc_ps in ((0, fcA_ps), (1, fcB_ps)):
                    for j in range(4):
                        mm = nc.tensor.matmul(
                            fc_ps[32 * j:32 * j + 1, :], one[:],
                            bfc[:, j * 1024 + bi * 512:j * 1024 + (bi + 1) * 512],
                            start=True, stop=False, tile_position=(0, 32 * j),
                            skip_group_check=True)
                        chain("pe_last", mm)
                    firstb = None
                    for c in range(8):
                        for j in range(4):
                            mm = nc.tensor.matmul(
                                fc_ps[32 * j:32 * j + 1, :],
                                h1buf[p][:, c:c + 1],
                                wfc[:, c * VLOC + j * 1024 + bi * 512:
                                    c * VLOC + j * 1024 + (bi + 1) * 512],
                                start=False, stop=(c == 7),
                                tile_position=(0, 32 * j),
                                skip_group_check=True)
                            chain("pe_last", mm)
                            if bi == 0 and firstb is None:
                                firstb = mm
                                if "comm" not in ABLATE:
                                    post_waits.append((mm, h1_sem, r_h1))
                    nc.scalar.activation(stg[0:97, bi * 512:(bi + 1) * 512],
                                         fc_ps[0:97, :], AF.Relu)

                # ---- local argmax over [4p, 1024]
                nc.vector.max(mx[0:97, :], stg[0:97, :])
                nc.vector.max_index(mi[0:97, :], mx[0:97, :], stg[0:97, :])
                nc.vector.tensor_copy(mif[0:97, :], mi[0:97, 0:1])
                nc.vector.tensor_tensor(gcand[0:97, 1:2], mif[0:97, :],
                                        base[0:97, :], ALU.add)
                nc.vector.tensor_copy(gcand[0:97, 0:1], mx[0:97, 0:1])
                trv = nc.tensor.transpose(ctv_ps[:], gcand[:, 0:1], ident[:])
                chain("pe_last", trv)
                tri = nc.tensor.transpose(cti_ps[:], gcand[:, 1:2], ident[:])
                chain("pe_last", tri)
                gv = gvs[u]
                nc.vector.tensor_reduce(gv[:, 0:1], ctv_ps[0:1, 0:97:32],
                                        mybir.AxisListType.X, ALU.max)
                nc.vector.tensor_tensor(gv[:, 1:5], ctv_ps[0:1, 0:97:32],
                                        gv[:, 0:1].to_broadcast((1, 4)),
                                        ALU.is_ge)
                nc.vector.scalar_tensor_tensor(
                    gv[:, 5:9], cti_ps[0:1, 0:97:32], -BIGVAL, gv[:, 1:5],
                    ALU.add, ALU.mult)
                nc.vector.tensor_scalar_add(gv[:, 5:9], gv[:, 5:9], BIGVAL)
                nc.vector.tensor_reduce(cdbuf[p][0:1, 1:2], gv[:, 5:9],
                                        mybir.AxisListType.X, ALU.min)
                nc.vector.tensor_copy(cdbuf[p][0:1, 0:1], gv[:, 0:1])
                bcast7(cdbuf[p], 2, cd_sem, cdbuf[p][:, 0:2])

                # ---- output row
                row = i_var * 2 + (u + 1)
                nc.sync.dma_start(out_d[bass.ds(row, 1), :], stg[P32, :])

                # ---- global argmax -> x for next step
                ra2 = nc.vector.reg_add(r_cd, r_cd, 14)
                chain("dve_last", ra2)
                glob = globs[u]
                rd = nc.vector.tensor_reduce(glob[:, 0:1],
                                             cdbuf[p][0:1, 0:16:2],
                                             mybir.AxisListType.X, ALU.max)
                chain("dve_last", rd)
                if "comm" not in ABLATE:
                    post_waits.append((rd, cd_sem, r_cd))
                nc.vector.tensor_tensor(glob[:, 1:9], cdbuf[p][0:1, 0:16:2],
                                        glob[:, 0:1].to_broadcast((1, 8)),
                                        ALU.is_ge)
                nc.vector.scalar_tensor_tensor(
                    glob[:, 9:17], cdbuf[p][0:1, 1:16:2], -BIGVAL, glob[:, 1:9],
                    ALU.add, ALU.mult)
                nc.vector.tensor_scalar_add(glob[:, 9:17], glob[:, 9:17], BIGVAL)
                nc.vector.tensor_reduce(x_s[:], glob[:, 9:17],
                                        mybir.AxisListType.X, ALU.min)

            # per-unroll scratch tiles
            stg_sh = sp.tile([128, 1024], F32, tag="stg", name="stg")
            stgs = [stg_sh, stg_sh]
            mxs = [sp.tile([128, 8], F32, tag=f"mx{u}", name=f"mx{u}") for u in range(2)]
            mis = [sp.tile([128, 8], U32, tag=f"mi{u}", name=f"mi{u}") for u in range(2)]
            mifs = [sp.tile([128, 1], F32, tag=f"mif{u}", name=f"mif{u}") for u in range(2)]
            gcands = [sp.tile([128, 2], F32, tag=f"gc{u}", name=f"gc{u}") for u in range(2)]
            gvs = [sp.tile([1, 9], F32, tag=f"gv{u}", name=f"gv{u}") for u in range(2)]
            globs = [sp.tile([1, 17], F32, tag=f"gl{u}", name=f"gl{u}") for u in range(2)]
            gates_sb = [sp.tile([128, 128], F32, tag=f"ga{u}", name=f"ga{u}") for u in range(2)]
            gates_sb2 = [sp.tile([128, 128], F32, tag=f"gb{u}", name=f"gb{u}") for u in range(2)]
            th_s = [sp.tile([128, 1], F32, tag=f"th{u}", name=f"th{u}") for u in range(2)]
            th2_s = [sp.tile([128, 1], F32, tag=f"th2{u}", name=f"th2{u}") for u in range(2)]
            t1_s = [sp.tile([128, 1], F32, tag=f"t1{u}", name=f"t1{u}") for u in range(2)]
            t2_s = [sp.tile([128, 1], F32, tag=f"t2{u}", name=f"t2{u}") for u in range(2)]

            with tc.For_i(0, n_iters, 1, hint_engines=(
                    mybir.EngineType.PE, mybir.EngineType.DVE,
                    mybir.EngineType.Activation, mybir.EngineType.Pool)) as i:
                step(0, i)
                step(1, i)

    for inst, sem, reg in post_waits:
        inst.wait_op(sem, reg, "sem-ge", check=False)
    nc.compile()
    return nc


def _prep_inputs(y, context_vector, w_ih0, w_hh0, b_ih0, b_hh0,
                 w_ih1, w_hh1, b_ih1, b_hh1, w_fc, b_fc):
    """Per-core input dicts implementing the sharding + permutations."""
    f32 = np.float32
    w_fc_pad = np.zeros((VPAD, H), dtype=f32)
    w_fc_pad[:V] = w_fc
    b_fc_pad = np.full(VPAD, -1.0e30, dtype=f32)
    b_fc_pad[:V] = b_fc

    b0_all = (b_ih0 + b_hh0).astype(f32)
    b1_all = (b_ih1 + b_hh1).astype(f32)

    in_maps = []
    for r in range(NCORES):
        rows = [GATE_OFF[j] + 128 * r + p for j in range(4) for p in range(128)]
        rows = np.array(rows)  # 512 gate rows of this core, tile-major

        def pack_w(w):  # w [4H, H] -> [128, 8*512] chunk-major, XOR-permuted
            out = np.empty((128, 8 * 512), dtype=f32)
            for c in range(8):
                src = eff_src(r, c)
                blk = w[rows, 128 * src:128 * (src + 1)]  # [512, 128]
                out[:, c * 512:(c + 1) * 512] = blk.T
            return out

        wfc_r = np.empty((128, 8 * VLOC), dtype=f32)
        for c in range(8):
            src = eff_src(r, c)
            blk = w_fc_pad[VLOC * r:VLOC * (r + 1), 128 * src:128 * (src + 1)]
            wfc_r[:, c * VLOC:(c + 1) * VLOC] = blk.T

        base_r = np.zeros((128, 1), dtype=f32)
        for j in range(4):
            base_r[32 * j, 0] = VLOC * r + 1024 * j

        def pack_h(hvec):  # full [H] -> [128, 8] by slot
            out = np.empty((128, 8), dtype=f32)
            for c in range(8):
                src = eff_src(r, c)
                out[:, c] = hvec[128 * src:128 * (src + 1)]
            return out

        in_maps.append({
            "wfc": wfc_r,
            "hh0": pack_w(w_hh0.astype(f32)),
            "ih1": pack_w(w_ih1.astype(f32)),
            "hh1": pack_w(w_hh1.astype(f32)),
            "wih0": w_ih0.astype(f32)[rows, 0].reshape(1, 512),
            "b0": b0_all[rows].reshape(1, 512),
            "b1": b1_all[rows].reshape(1, 512),
            "bfc": b_fc_pad[VLOC * r:VLOC * (r + 1)].reshape(1, VLOC),
            "base": base_r,
            "h0init": pack_h(context_vector[0].astype(f32)),
            "h1init": pack_h(context_vector[1].astype(f32)),
            "c0init": context_vector[0].astype(f32)[128 * r:128 * (r + 1)].reshape(128, 1),
            "c1init": context_vector[1].astype(f32)[128 * r:128 * (r + 1)].reshape(128, 1),
            "x0": np.array([[np.float32(y[0])]], dtype=f32),
        })
    return in_maps


_CACHED = {}


def _get_nc(n_iters):
    if n_iters not in _CACHED:
        _CACHED[n_iters] = build_decoder(n_iters)
    return _CACHED[n_iters]


def kernel(y, context_vector, w_ih0, w_hh0, b_ih0, b_hh0,
           w_ih1, w_hh1, b_ih1, b_hh1, w_fc, b_fc):
    from concourse import bass_utils

    n_iters = L // 2  # 1024 iterations x 2 steps = 2048 steps; rows 1..2047 used
    nc = _get_nc(n_iters)
    in_maps = _prep_inputs(np.asarray(y), np.asarray(context_vector),
                           np.asarray(w_ih0), np.asarray(w_hh0),
                           np.asarray(b_ih0), np.asarray(b_hh0),
                           np.asarray(w_ih1), np.asarray(w_hh1),
                           np.asarray(b_ih1), np.asarray(b_hh1),
                           np.asarray(w_fc), np.asarray(b_fc))
    res = bass_utils.run_bass_kernel_spmd(nc, in_maps,
                                          core_ids=list(range(NCORES)))
    global LAST_EXEC_NS, LAST_TRACE
    LAST_EXEC_NS = getattr(res, "exec_time_ns", None)
    it = getattr(res, "instructions_and_trace", None)
    LAST_TRACE = it[1] if it else None
    out = np.zeros((L, V), dtype=np.float32)
    for r in range(NCORES):
        o = res.results[r]["out"]  # [2*n_iters+1, VLOC]
        lo = VLOC * r
        hi = min(VLOC * (r + 1), V)
        if lo < V:
            out[1:L, lo:hi] = o[1:L, :hi - lo]
    return out



# revision 8
# speedup vs baseline: 1.0626x; 1.0626x over previous
"""Trainium2 Bass kernel for nn_Decoder: 2-layer LSTM + vocab-32000 greedy decoder.

Strategy (8 NeuronCores, one trn2 chip):
- Everything fp32 on the PE (exact argmax tracking vs the fp32 reference).
- All matvecs run "weights-moving": stationary = h columns [128,1], moving =
  W^T chunks streamed at N columns/instr, 4-way col-tiled for 4x concurrency.
- fc weight [32768, 1024] sharded by vocab across cores (4096 rows/core,
  resident in SBUF).  LSTM hidden state sharded 128 units/core; per step the
  h-slices are all-gathered via remote SBUF-to-SBUF DMA (XOR slot pattern),
  the per-core argmax candidates likewise.  3 small exchanges per step.
- Single NEFF, 1024-iteration For_i loop x 2 unrolled steps = 2048 steps.
"""
import numpy as np

import concourse.bass as bass
import concourse.mybir as mybir
import concourse.tile as tile
from concourse import bacc
from concourse.bass import _add_dep_helper
from concourse.masks import make_identity

F32 = mybir.dt.float32
U32 = mybir.dt.uint32
AF = mybir.ActivationFunctionType
ALU = mybir.AluOpType

H = 1024
V = 32000
VPAD = 32768
VLOC = VPAD // 8          # 4096 vocab rows per core
L = 2048
NCORES = 8
# logical -> physical NeuronCore map observed on this trn2 chip (involution).
PERM = [0, 1, 2, 3, 6, 7, 4, 5]
BIGVAL = 65536.0
ABLATE = set()
UNROLL = 8  # steps per For_i iteration (even)  # > any vocab idx; keeps idx arithmetic exact in fp32

# torch gate row order in the 4H weights: i, f, g, o.
# col-tile j holds gate type: j=0 -> i, 1 -> f, 2 -> o, 3 -> g
GATE_OFF = [0, H, 3 * H, 2 * H]  # row offset of gate-type j in the 4H dim


def eff_src(r, c):
    """Logical id of the core whose data lands in receiver r's slot c."""
    return PERM[PERM[r] ^ c]


def build_decoder(n_iters, out_rows=None, fixed_row=False):
    """Build the SPMD program. n_iters loop iterations x 2 steps each.

    fixed_row=True writes every step's output to row 1 (timing builds only:
    keeps DMA cost identical while allowing huge n_iters with small out_d).
    """
    if out_rows is None:
        out_rows = UNROLL * n_iters + 1
    nc = bacc.Bacc(None, num_devices=NCORES, detect_race_conditions=False)

    wfc_d = nc.dram_tensor("wfc", [128, 8 * VLOC], F32, kind="ExternalInput")
    hh0_d = nc.dram_tensor("hh0", [128, 8 * 512], F32, kind="ExternalInput")
    ih1_d = nc.dram_tensor("ih1", [128, 8 * 512], F32, kind="ExternalInput")
    hh1_d = nc.dram_tensor("hh1", [128, 8 * 512], F32, kind="ExternalInput")
    wih0_d = nc.dram_tensor("wih0", [1, 512], F32, kind="ExternalInput")
    b0_d = nc.dram_tensor("b0", [1, 512], F32, kind="ExternalInput")
    b1_d = nc.dram_tensor("b1", [1, 512], F32, kind="ExternalInput")
    bfc_d = nc.dram_tensor("bfc", [1, VLOC], F32, kind="ExternalInput")
    base_d = nc.dram_tensor("base", [128, 1], F32, kind="ExternalInput")
    h0i_d = nc.dram_tensor("h0init", [128, 8], F32, kind="ExternalInput")
    h1i_d = nc.dram_tensor("h1init", [128, 8], F32, kind="ExternalInput")
    c0i_d = nc.dram_tensor("c0init", [128, 1], F32, kind="ExternalInput")
    c1i_d = nc.dram_tensor("c1init", [128, 1], F32, kind="ExternalInput")
    x0_d = nc.dram_tensor("x0", [1, 1], F32, kind="ExternalInput")
    out_d = nc.dram_tensor("out", [out_rows, VLOC], F32,
                           kind="ExternalOutput")

    h0_sem = nc.alloc_semaphore("h0_sem")
    h1_sem = nc.alloc_semaphore("h1_sem")
    cd_sem = nc.alloc_semaphore("cd_sem")
    lsem = nc.alloc_semaphore("lsem")
    nc.add_non_barrier_sems([h0_sem.num, h1_sem.num, cd_sem.num, lsem.num])

    r_h0 = nc.tensor.alloc_register("r_h0")
    r_h1 = nc.tensor.alloc_register("r_h1")
    r_cd = nc.vector.alloc_register("r_cd")

    post_waits = []   # (instruction, sem, register)
    P32 = slice(0, 97, 32)   # partitions {0,32,64,96}

    with tile.TileContext(nc) as tc:
        with tc.tile_pool(name="wts", bufs=1) as wp, \
             tc.tile_pool(name="st", bufs=1) as sp, \
             tc.tile_pool(name="ps", bufs=1, space="PSUM") as pp:

            wfc = wp.tile([128, 8 * VLOC], F32, tag="wfc")
            hh0 = wp.tile([128, 8 * 512], F32, tag="hh0")
            ih1 = wp.tile([128, 8 * 512], F32, tag="ih1")
            hh1 = wp.tile([128, 8 * 512], F32, tag="hh1")
            wih0 = wp.tile([1, 512], F32, tag="wih0")
            b0 = wp.tile([1, 512], F32, tag="b0")
            b1 = wp.tile([1, 512], F32, tag="b1")
            bfc = wp.tile([1, VLOC], F32, tag="bfc")
            base = wp.tile([128, 1], F32, tag="base")
            ident = wp.tile([128, 128], F32, tag="ident")
            one = wp.tile([1, 1], F32, tag="one")
            onecol = wp.tile([128, 1], F32, tag="onecol")
            big4 = wp.tile([1, 4], F32, tag="big4")
            big8 = wp.tile([1, 8], F32, tag="big8")
            x_s = wp.tile([1, 1], F32, tag="x")
            c0 = wp.tile([128, 1], F32, tag="c0")
            gcol_s = wp.tile([128, 1], F32, tag="gcol")
            c1 = wp.tile([128, 1], F32, tag="c1")
            h0buf = [wp.tile([128, 8], F32, tag=f"h0buf{p}", name=f"h0buf{p}")
                     for p in range(2)]
            h1buf = [wp.tile([128, 8], F32, tag=f"h1buf{p}", name=f"h1buf{p}")
                     for p in range(2)]
            cdbuf = [wp.tile([128, 16], F32, tag=f"cdbuf{p}", name=f"cdbuf{p}")
                     for p in range(2)]

            for dst, src in ((wfc, wfc_d), (hh0, hh0_d), (ih1, ih1_d),
                             (hh1, hh1_d), (wih0, wih0_d), (b0, b0_d),
                             (b1, b1_d), (bfc, bfc_d), (base, base_d),
                             (h0buf[1], h0i_d), (h1buf[1], h1i_d),
                             (c0, c0i_d), (c1, c1i_d), (x_s, x0_d)):
                nc.sync.dma_start(dst[:], src[:])
            make_identity(nc, ident[:])
            nc.vector.memset(h0buf[0][:], 0.0)
            nc.vector.memset(h1buf[0][:], 0.0)
            nc.vector.memset(cdbuf[0][:], 0.0)
            nc.vector.memset(cdbuf[1][:], 0.0)
            nc.vector.memset(one[:], 1.0)
            nc.vector.memset(onecol[:], 1.0)
            nc.vector.memset(big4[:], BIGVAL)
            nc.vector.memset(big8[:], BIGVAL)
            rm0 = nc.tensor.reg_mov(r_h0, 0)
            rm1 = nc.tensor.reg_mov(r_h1, 0)
            rm2 = nc.vector.reg_mov(r_cd, 0)

            # psum tiles (8 banks):
            g0_ps = pp.tile([128, 128], F32, tag="g0")
            g1_ps = pp.tile([128, 128], F32, tag="g1")
            tr_ps = pp.tile([128, 128], F32, tag="tr")
            fcA_ps = pp.tile([128, 512], F32, tag="fcA")
            fcB_ps = pp.tile([128, 512], F32, tag="fcB")
            ctv_ps = pp.tile([1, 128], F32, tag="ctv")
            cti_ps = pp.tile([1, 128], F32, tag="cti")
            for _pst in (g0_ps, g1_ps, fcA_ps, fcB_ps):
                nc.vector.memset(_pst[:], 0.0)

            state = {
                "pe_last": rm1, "dve_last": rm2,
                "prep_last": None, "trig_last": None,
            }

            def chain(engine_key, inst):
                prev = state[engine_key]
                if prev is not None:
                    _add_dep_helper(inst.ins, prev.ins, sync=False,
                                    reason=f"order {engine_key}")
                state[engine_key] = inst
                return inst

            def bcast7(buf, width, sem, src_ap):
                """7 broadcasts of src_ap into peers' buf slot k, then trigger."""
                if "comm" in ABLATE:
                    return None
                for k in range(1, 8):
                    rdests = [None] * 8
                    rdests[k] = (0, k)
                    pr = nc.gpsimd.remote_dma_broadcast(
                        buf[:, k * width:(k + 1) * width], src_ap,
                        sem, lsem, rdests=rdests)
                    chain("prep_last", pr)
                tg = nc.gpsimd.trigger_dma(count=7)
                chain("prep_last", tg)
                return tg

            def cell(l_idx, g_ps, gate_sb, c_st, th_t, t1, t2, hdst):
                """LSTM cell: gates psum [4p,128] -> h column [128,1].

                One sigmoid ACT over all bands; the g-band weights/bias are
                pre-doubled so tanh(g) = 2*sigmoid(2g) - 1 (DVE affine).
                """
                nc.scalar.activation(gate_sb[0:97, 0:128],
                                     g_ps[0:97, 0:128], AF.Sigmoid)
                tr = nc.tensor.transpose(tr_ps[:], gate_sb[:], ident[:])
                chain("pe_last", tr)
                # cols after transpose: i@0, f@32, o@64, sigma(2g)@96
                nc.vector.scalar_tensor_tensor(
                    gcol_s[:], tr_ps[:, 96:97], 2.0, onecol[:],
                    ALU.mult, ALU.subtract)
                nc.vector.tensor_tensor(t1[:], tr_ps[:, 0:1], gcol_s[:],
                                        ALU.mult)
                nc.vector.tensor_tensor(t2[:], tr_ps[:, 32:33], c_st[:],
                                        ALU.mult)
                nc.vector.tensor_tensor(c_st[:], t1[:], t2[:], ALU.add)
                nc.scalar.activation(th_t[:], c_st[:], AF.Tanh)
                nc.vector.tensor_tensor(hdst, tr_ps[:, 64:65], th_t[:],
                                        ALU.mult)

            def step(u, i_var):
                v = u % 2
                p, q = v, 1 - v
                stg = stgs[v]
                mx, mi, mif, gcand = mxs[v], mis[v], mifs[v], gcands[v]

                # ---- g0 = b0 + hh0 @ h0(q) + x*wih0
                for j in range(4):
                    mm = nc.tensor.matmul(
                        g0_ps[32 * j:32 * j + 1, 0:128], one[:],
                        b0[:, j * 128:(j + 1) * 128],
                        start=True, stop=False, tile_position=(0, 32 * j),
                        skip_group_check=True)
                    chain("pe_last", mm)
                for c in range(8):
                    for j in range(4):
                        mm = nc.tensor.matmul(
                            g0_ps[32 * j:32 * j + 1, 0:128],
                            h0buf[q][:, c:c + 1],
                            hh0[:, c * 512 + j * 128:c * 512 + (j + 1) * 128],
                            start=False, stop=False, tile_position=(0, 32 * j),
                            skip_group_check=True)
                        chain("pe_last", mm)
                for j in range(4):
                    mm = nc.tensor.matmul(
                        g0_ps[32 * j:32 * j + 1, 0:128], x_s[:],
                        wih0[:, j * 128:(j + 1) * 128],
                        start=False, stop=(j == 3), tile_position=(0, 32 * j),
                        skip_group_check=True)
                    chain("pe_last", mm)

                # ---- cell0 -> h0 column into slot 0 of h0buf[p], broadcast
                cell(0, g0_ps, gates_sb[v], c0, th_s[v], t1_s[v], t2_s[v],
                     h0buf[p][:, 0:1])
                bcast7(h0buf[p], 1, h0_sem, h0buf[p][:, 0:1])

                # ---- g1 = b1 + hh1 @ h1(q) + ih1 @ h0(p)
                for j in range(4):
                    mm = nc.tensor.matmul(
                        g1_ps[32 * j:32 * j + 1, 0:128], one[:],
                        b1[:, j * 128:(j + 1) * 128],
                        start=True, stop=False, tile_position=(0, 32 * j),
                        skip_group_check=True)
                    chain("pe_last", mm)
                for c in range(8):
                    for j in range(4):
                        mm = nc.tensor.matmul(
                            g1_ps[32 * j:32 * j + 1, 0:128],
                            h1buf[q][:, c:c + 1],
                            hh1[:, c * 512 + j * 128:c * 512 + (j + 1) * 128],
                            start=False, stop=False, tile_position=(0, 32 * j),
                            skip_group_check=True)
                        chain("pe_last", mm)
                ra = nc.tensor.reg_add(r_h0, r_h0, 14)
                chain("pe_last", ra)
                first = None
                for c in range(8):
                    for j in range(4):
                        mm = nc.tensor.matmul(
                            g1_ps[32 * j:32 * j + 1, 0:128],
                            h0buf[p][:, c:c + 1],
                            ih1[:, c * 512 + j * 128:c * 512 + (j + 1) * 128],
                            start=False, stop=(c == 7), tile_position=(0, 32 * j),
                            skip_group_check=True)
                        chain("pe_last", mm)
                        if first is None:
                            first = mm
                            if "comm" not in ABLATE:
                                post_waits.append((mm, h0_sem, r_h0))

                # ---- cell1 -> h1 column, broadcast
                cell(1, g1_ps, gates_sb2[v], c1, th2_s[v], t1_s[v], t2_s[v],
                     h1buf[p][:, 0:1])
                bcast7(h1buf[p], 1, h1_sem, h1buf[p][:, 0:1])

                # ---- fc = relu(bfc + Wfc @ h1(p)) ; two banks per col-tile
                ra1 = nc.tensor.reg_add(r_h1, r_h1, 14)
                chain("pe_last", ra1)
                for bi, fc_ps in ((0, fcA_ps), (1, fcB_ps)):
                    for j in range(4):
                        mm = nc.tensor.matmul(
                            fc_ps[32 * j:32 * j + 1, :], one[:],
                            bfc[:, j * 1024 + bi * 512:j * 1024 + (bi + 1) * 512],
                            start=True, stop=False, tile_position=(0, 32 * j),
                            skip_group_check=True)
                        chain("pe_last", mm)
                    firstb = None
                    for c in range(8):
                        for j in range(4):
                            mm = nc.tensor.matmul(
                                fc_ps[32 * j:32 * j + 1, :],
                                h1buf[p][:, c:c + 1],
                                wfc[:, c * VLOC + j * 1024 + bi * 512:
                                    c * VLOC + j * 1024 + (bi + 1) * 512],
                                start=False, stop=(c == 7),
                                tile_position=(0, 32 * j),
                                skip_group_check=True)
                            chain("pe_last", mm)
                            if bi == 0 and firstb is None:
                                firstb = mm
                                if "comm" not in ABLATE:
                                    post_waits.append((mm, h1_sem, r_h1))
                    nc.scalar.activation(stg[0:97, bi * 512:(bi + 1) * 512],
                                         fc_ps[0:97, :], AF.Relu)

                # ---- local argmax over [4p, 1024]
                nc.vector.max(mx[0:97, :], stg[0:97, :])
                nc.vector.max_index(mi[0:97, :], mx[0:97, :], stg[0:97, :])
                nc.vector.tensor_copy(mif[0:97, :], mi[0:97, 0:1])
                nc.vector.tensor_tensor(gcand[0:97, 1:2], mif[0:97, :],
                                        base[0:97, :], ALU.add)
                nc.vector.tensor_copy(gcand[0:97, 0:1], mx[0:97, 0:1])
                trv = nc.tensor.transpose(ctv_ps[:], gcand[:, 0:1], ident[:])
                chain("pe_last", trv)
                tri = nc.tensor.transpose(cti_ps[:], gcand[:, 1:2], ident[:])
                chain("pe_last", tri)
                gv = gvs[v]
                nc.vector.tensor_reduce(gv[:, 0:1], ctv_ps[0:1, 0:97:32],
                                        mybir.AxisListType.X, ALU.max)
                nc.vector.tensor_tensor(gv[:, 1:5], ctv_ps[0:1, 0:97:32],
                                        gv[:, 0:1].to_broadcast((1, 4)),
                                        ALU.is_ge)
                nc.vector.scalar_tensor_tensor(
                    gv[:, 5:9], cti_ps[0:1, 0:97:32], -BIGVAL, gv[:, 1:5],
                    ALU.add, ALU.mult)
                nc.vector.tensor_scalar_add(gv[:, 5:9], gv[:, 5:9], BIGVAL)
                nc.vector.tensor_reduce(cdbuf[p][0:1, 1:2], gv[:, 5:9],
                                        mybir.AxisListType.X, ALU.min)
                nc.vector.tensor_copy(cdbuf[p][0:1, 0:1], gv[:, 0:1])
                bcast7(cdbuf[p], 2, cd_sem, cdbuf[p][:, 0:2])

                # ---- output row
                if "outdma" not in ABLATE:
                    if fixed_row:
                        nc.sync.dma_start(out_d[bass.ds(1, 1), :], stg[P32, :])
                    else:
                        row = i_var * UNROLL + (u + 1)
                        nc.sync.dma_start(out_d[bass.ds(row, 1), :], stg[P32, :])

                # ---- global argmax -> x for next step
                ra2 = nc.vector.reg_add(r_cd, r_cd, 14)
                chain("dve_last", ra2)
                glob = globs[v]
                rd = nc.vector.tensor_reduce(glob[:, 0:1],
                                             cdbuf[p][0:1, 0:16:2],
                                             mybir.AxisListType.X, ALU.max)
                chain("dve_last", rd)
                if "comm" not in ABLATE:
                    post_waits.append((rd, cd_sem, r_cd))
                nc.vector.tensor_tensor(glob[:, 1:9], cdbuf[p][0:1, 0:16:2],
                                        glob[:, 0:1].to_broadcast((1, 8)),
                                        ALU.is_ge)
                nc.vector.scalar_tensor_tensor(
                    glob[:, 9:17], cdbuf[p][0:1, 1:16:2], -BIGVAL, glob[:, 1:9],
                    ALU.add, ALU.mult)
                nc.vector.tensor_scalar_add(glob[:, 9:17], glob[:, 9:17], BIGVAL)
                nc.vector.tensor_reduce(x_s[:], glob[:, 9:17],
                                        mybir.AxisListType.X, ALU.min)

            # per-unroll scratch tiles
            stg_sh = sp.tile([128, 1024], F32, tag="stg", name="stg")
            stgs = [stg_sh, stg_sh]
            mxs = [sp.tile([128, 8], F32, tag=f"mx{u}", name=f"mx{u}") for u in range(2)]
            mis = [sp.tile([128, 8], U32, tag=f"mi{u}", name=f"mi{u}") for u in range(2)]
            mifs = [sp.tile([128, 1], F32, tag=f"mif{u}", name=f"mif{u}") for u in range(2)]
            gcands = [sp.tile([128, 2], F32, tag=f"gc{u}", name=f"gc{u}") for u in range(2)]
            gvs = [sp.tile([1, 9], F32, tag=f"gv{u}", name=f"gv{u}") for u in range(2)]
            globs = [sp.tile([1, 17], F32, tag=f"gl{u}", name=f"gl{u}") for u in range(2)]
            gates_sb = [sp.tile([128, 128], F32, tag=f"ga{u}", name=f"ga{u}") for u in range(2)]
            gates_sb2 = [sp.tile([128, 128], F32, tag=f"gb{u}", name=f"gb{u}") for u in range(2)]
            th_s = [sp.tile([128, 1], F32, tag=f"th{u}", name=f"th{u}") for u in range(2)]
            th2_s = [sp.tile([128, 1], F32, tag=f"th2{u}", name=f"th2{u}") for u in range(2)]
            t1_s = [sp.tile([128, 1], F32, tag=f"t1{u}", name=f"t1{u}") for u in range(2)]
            t2_s = [sp.tile([128, 1], F32, tag=f"t2{u}", name=f"t2{u}") for u in range(2)]

            with tc.For_i(0, n_iters, 1, hint_engines=(
                    mybir.EngineType.PE, mybir.EngineType.DVE,
                    mybir.EngineType.Activation, mybir.EngineType.Pool)) as i:
                for u in range(UNROLL):
                    step(u, i)

    for inst, sem, reg in post_waits:
        inst.wait_op(sem, reg, "sem-ge", check=False)
    nc.compile()
    return nc


def _prep_inputs(y, context_vector, w_ih0, w_hh0, b_ih0, b_hh0,
                 w_ih1, w_hh1, b_ih1, b_hh1, w_fc, b_fc):
    """Per-core input dicts implementing the sharding + permutations."""
    f32 = np.float32
    w_fc_pad = np.zeros((VPAD, H), dtype=f32)
    w_fc_pad[:V] = w_fc
    b_fc_pad = np.full(VPAD, -1.0e30, dtype=f32)
    b_fc_pad[:V] = b_fc

    def _x2g(a):  # double the g-band (cols 384:512) of a [1,512] row
        a = a.copy()
        a[0, 384:] *= 2.0
        return a

    b0_all = (b_ih0 + b_hh0).astype(f32)
    b1_all = (b_ih1 + b_hh1).astype(f32)

    in_maps = []
    for r in range(NCORES):
        rows = [GATE_OFF[j] + 128 * r + p for j in range(4) for p in range(128)]
        rows = np.array(rows)  # 512 gate rows of this core, tile-major

        def pack_w(w):  # w [4H, H] -> [128, 8*512] chunk-major, XOR-permuted
            # g-band (col-tile 3) doubled for the 2*sigmoid(2g)-1 identity
            out = np.empty((128, 8 * 512), dtype=f32)
            for c in range(8):
                src = eff_src(r, c)
                blk = w[rows, 128 * src:128 * (src + 1)]  # [512, 128]
                out[:, c * 512:(c + 1) * 512] = blk.T
                out[:, c * 512 + 384:(c + 1) * 512] *= 2.0
            return out

        wfc_r = np.empty((128, 8 * VLOC), dtype=f32)
        for c in range(8):
            src = eff_src(r, c)
            blk = w_fc_pad[VLOC * r:VLOC * (r + 1), 128 * src:128 * (src + 1)]
            wfc_r[:, c * VLOC:(c + 1) * VLOC] = blk.T

        base_r = np.zeros((128, 1), dtype=f32)
        for j in range(4):
            base_r[32 * j, 0] = VLOC * r + 1024 * j

        def pack_h(hvec):  # full [H] -> [128, 8] by slot
            out = np.empty((128, 8), dtype=f32)
            for c in range(8):
                src = eff_src(r, c)
                out[:, c] = hvec[128 * src:128 * (src + 1)]
            return out

        in_maps.append({
            "wfc": wfc_r,
            "hh0": pack_w(w_hh0.astype(f32)),
            "ih1": pack_w(w_ih1.astype(f32)),
            "hh1": pack_w(w_hh1.astype(f32)),
            "wih0": _x2g(w_ih0.astype(f32)[rows, 0].reshape(1, 512)),
            "b0": _x2g(b0_all[rows].reshape(1, 512)),
            "b1": _x2g(b1_all[rows].reshape(1, 512)),
            "bfc": b_fc_pad[VLOC * r:VLOC * (r + 1)].reshape(1, VLOC),
            "base": base_r,
            "h0init": pack_h(context_vector[0].astype(f32)),
            "h1init": pack_h(context_vector[1].astype(f32)),
            "c0init": context_vector[0].astype(f32)[128 * r:128 * (r + 1)].reshape(128, 1),
            "c1init": context_vector[1].astype(f32)[128 * r:128 * (r + 1)].reshape(128, 1),
            "x0": np.array([[np.float32(y[0])]], dtype=f32),
        })
    return in_maps


_CACHED = {}


def _get_nc(n_iters):
    if n_iters not in _CACHED:
        _CACHED[n_iters] = build_decoder(n_iters)
    return _CACHED[n_iters]


def kernel(y, context_vector, w_ih0, w_hh0, b_ih0, b_hh0,
           w_ih1, w_hh1, b_ih1, b_hh1, w_fc, b_fc):
    from concourse import bass_utils

    n_iters = L // UNROLL  # UNROLL steps per iteration; rows 1..2047 used
    nc = _get_nc(n_iters)
    in_maps = _prep_inputs(np.asarray(y), np.asarray(context_vector),
                           np.asarray(w_ih0), np.asarray(w_hh0),
                           np.asarray(b_ih0), np.asarray(b_hh0),
                           np.asarray(w_ih1), np.asarray(w_hh1),
                           np.asarray(b_ih1), np.asarray(b_hh1),
                           np.asarray(w_fc), np.asarray(b_fc))
    res = bass_utils.run_bass_kernel_spmd(nc, in_maps,
                                          core_ids=list(range(NCORES)))
    global LAST_EXEC_NS, LAST_TRACE
    LAST_EXEC_NS = getattr(res, "exec_time_ns", None)
    it = getattr(res, "instructions_and_trace", None)
    LAST_TRACE = it[1] if it else None
    out = np.zeros((L, V), dtype=np.float32)
    for r in range(NCORES):
        o = res.results[r]["out"]  # [2*n_iters+1, VLOC]
        lo = VLOC * r
        hi = min(VLOC * (r + 1), V)
        if lo < V:
            out[1:L, lo:hi] = o[1:L, :hi - lo]
    return out

